# revision 5
# baseline (speedup 1.0000x reference)
"""Malvar-He-Cutler demosaic as a Trainium2 Bass kernel.

Strategy (per core; batch 16 is sharded 2 images/core across 8 cores):
  - The 5x5 conv producing 4 feature maps is evaluated per output-pixel
    parity class (Bayer quadrant).  For each of the 8 needed
    (feature, quadrant) planes the 2D conv is expressed as a short sum of
    banded-matrix matmuls on the TensorEngine: contraction runs over image
    rows held in SBUF partitions (row-parity-split tiles with a 1-quad-row
    halo), while column taps become stride-2 access-pattern offsets of the
    moving operand.
  - fp32 accuracy at bf16-class speed: x is split on-device into
    xh = fp16(x) (11-bit) and xl = x - xh (written as float32r, which the
    PE consumes at 1 cycle/row for free-dim >= 256).  The 5x5 coefficients
    (multiples of 1/16) are exact in fp16, so conv(x) = conv(xh) + conv(xl)
    accumulated in fp32 PSUM reproduces the fp32 reference to ~1e-7.
  - ScalarE/VectorE interleave the quarter-res planes back into full-res
    output rows in SBUF; rows DMA out contiguously.
"""

import os
import sys

import numpy as np

for _p in ("/opt/trn_rl_repo", "/root/.axon_site/_ro/trn_rl_repo"):
    if os.path.isdir(_p) and _p not in sys.path:
        sys.path.insert(0, _p)

import concourse.bacc as bacc
import concourse.mybir as mybir
import concourse.tile as tile

# ---------------------------------------------------------------- constants
_K = [
    0, 0, -2, 0, 0,  0, 0, 4, 0, 0,  -2, 4, 8, 4, -2,  0, 0, 4, 0, 0,  0, 0, -2, 0, 0,
    0, 0, -3, 0, 0,  0, 4, 0, 4, 0,  -3, 0, 12, 0, -3,  0, 4, 0, 4, 0,  0, 0, -3, 0, 0,
    0, 0, 1, 0, 0,  0, -2, 0, -2, 0,  -2, 8, 10, 8, -2,  0, -2, 0, -2, 0,  0, 0, 1, 0, 0,
    0, 0, -2, 0, 0,  0, -2, 8, -2, 0,  1, 0, 10, 0, 1,  0, -2, 8, -2, 0,  0, 0, -2, 0, 0,
]
KER = np.asarray(_K, dtype=np.float64).reshape(4, 5, 5) / 16.0
INDICES_RGGB = np.array([4, 2, 3, 1, 0, 4, 4, 0, 1, 3, 2, 4]).reshape(1, 3, 2, 2)

H = W = 1024
QH = H // 2          # quad rows per image
IMGS_PER_CORE = 2
N_CORES = 8
MBLK = 124           # output quad rows per block
NFREE = W // 2       # matmul free dim = quad cols
KPART = 126          # rhs partitions: 0..124 quads i0..i0+124, 125 = quad i0-1


def _calc_index(pattern):
    p = tuple(np.asarray(pattern).flatten().tolist())
    if p == (0, 1, 1, 2):
        return INDICES_RGGB
    if p == (1, 0, 2, 1):
        return np.roll(INDICES_RGGB, 1, axis=-1)
    if p == (1, 2, 0, 1):
        return np.roll(INDICES_RGGB, 1, axis=-2)
    if p == (2, 1, 1, 0):
        return np.roll(np.roll(INDICES_RGGB, 1, axis=-1), 1, axis=-2)
    raise ValueError("Invalid bayer pattern")


def _matmul_groups(k, a, b):
    """Group the nonzero taps of kernel k for output quadrant (a, b) by
    (source row-parity q, E-tile column offset).  Each group is one banded
    matmul; bands map tap quad-row offset d in {-1,0,1} to partitions."""
    groups = {}
    for dy in range(-2, 3):
        for dx in range(-2, 3):
            c = KER[k, dy + 2, dx + 2]
            if c == 0.0:
                continue
            q = (a + dy) % 2
            d = (a + dy - q) // 2
            coff = b + dx + 2
            bands = groups.setdefault((q, coff), {})
            bands[d] = bands.get(d, 0.0) + c
    return groups


def _bmat(bands):
    B = np.zeros((KPART, MBLK), np.float32)
    for mm in range(MBLK):
        for d, c in bands.items():
            p = mm + d
            if p == -1:
                p = 125
            B[p, mm] = c
    return B


def _build_plan(index):
    """index: (3,2,2).  Returns (planes, bmats) where planes is a list of
    (c, a, b, kind, payload): kind 'ident' (payload None) or 'conv'
    (payload = list of (bmat_idx, q, coff))."""
    bmats = []
    bkey = {}
    planes = []
    for c in range(3):
        for a in range(2):
            for b in range(2):
                k = int(index[c, a, b])
                if k == 4:
                    planes.append((c, a, b, "ident", None))
                    continue
                groups = _matmul_groups(k, a, b)
                glist = []
                for (q, coff), bands in sorted(groups.items()):
                    key = tuple(sorted((d, round(v * 16)) for d, v in bands.items()))
                    if key not in bkey:
                        bkey[key] = len(bmats)
                        bmats.append(_bmat(bands))
                    glist.append((bkey[key], q, coff))
                planes.append((c, a, b, "conv", glist))
    return planes, np.stack(bmats)


# ------------------------------------------------------------ bass program
def build_nc(planes, n_bmats, reps=1, two_pass=True):
    f32, f16, f32r = mybir.dt.float32, mybir.dt.float16, mybir.dt.float32r
    nc = bacc.Bacc("TRN2", target_bir_lowering=False, debug=False)
    x_d = nc.dram_tensor("x", [IMGS_PER_CORE, H, W], f32, kind="ExternalInput")
    bmh_d = nc.dram_tensor("bm_h", [n_bmats, KPART, MBLK], f16, kind="ExternalInput")
    bml_d = nc.dram_tensor("bm_l", [n_bmats, KPART, MBLK], f32r, kind="ExternalInput")
    y_d = nc.dram_tensor("y", [IMGS_PER_CORE, 3, H, W], f32, kind="ExternalOutput")

    i0s = list(range(0, QH, MBLK))  # block starts

    with tile.TileContext(nc) as tc:
        with (
            tc.tile_pool(name="consts", bufs=1) as cpool,
            tc.tile_pool(name="esrc", bufs=2) as epool,
            tc.tile_pool(name="stage", bufs=2) as spool,
            tc.tile_pool(name="psum", bufs=8, space="PSUM") as ppool,
        ):
            bh = []
            bl = []
            for i in range(n_bmats):
                th = cpool.tile([KPART, MBLK], f16, tag=f"bh{i}", name=f"bh{i}")
                nc.sync.dma_start(th[:], bmh_d[i])
                bh.append(th)
                tl = cpool.tile([KPART, MBLK], f32r, tag=f"bl{i}", name=f"bl{i}")
                nc.sync.dma_start(tl[:], bml_d[i])
                bl.append(tl)

            for rep in range(reps):
                for img in range(IMGS_PER_CORE):
                    for bi, i0 in enumerate(i0s):
                        m = min(MBLK, QH - i0)  # output quad rows this block
                        nvalid = min(125, QH - i0)
                        ragged = nvalid < 125
                        first_use = rep == 0 and img == 0 and bi < 2

                        E, Eh, El = {}, {}, {}
                        for q in range(2):
                            e = epool.tile([KPART, W + 4], f32, tag=f"E{q}", name=f"E{q}_{img}_{bi}")
                            if ragged or first_use:
                                nc.gpsimd.memset(e[:], 0.0)
                            # interior quad rows i0..i0+nvalid-1
                            nc.sync.dma_start(
                                e[0:nvalid, 2 : 2 + W],
                                x_d[img, 2 * i0 + q : 2 * (i0 + nvalid) + q - 1 : 2, :],
                            )
                            if ragged:  # quad QH -> clamp to last row
                                nc.sync.dma_start(
                                    e[nvalid : nvalid + 1, 2 : 2 + W],
                                    x_d[img, H - 1 : H, :],
                                )
                            # halo-above quad i0-1 at partition 125
                            hrow = 0 if i0 == 0 else 2 * (i0 - 1) + q
                            nc.sync.dma_start(
                                e[125:126, 2 : 2 + W], x_d[img, hrow : hrow + 1, :]
                            )
                            # horizontal replication pad
                            nc.gpsimd.tensor_copy(e[:, 0:1], e[:, 2:3])
                            nc.gpsimd.tensor_copy(e[:, 1:2], e[:, 2:3])
                            nc.gpsimd.tensor_copy(e[:, W + 2 : W + 3], e[:, W + 1 : W + 2])
                            nc.gpsimd.tensor_copy(e[:, W + 3 : W + 4], e[:, W + 1 : W + 2])
                            E[q] = e
                            eh = epool.tile([KPART, W + 4], f16, tag=f"Eh{q}", name=f"Eh{q}_{img}_{bi}")
                            nc.gpsimd.tensor_copy(eh[:], e[:])
                            Eh[q] = eh
                            if two_pass:
                                el = epool.tile([KPART, W + 4], f32r, tag=f"El{q}", name=f"El{q}_{img}_{bi}")
                                nc.vector.tensor_tensor(
                                    el[:], e[:], eh[:], mybir.AluOpType.subtract
                                )
                                El[q] = el

                        stg = {}
                        for cch in range(3):
                            for a in range(2):
                                stg[(cch, a)] = spool.tile([MBLK, W], f32, tag=f"st{cch}{a}", name=f"st{cch}{a}_{img}_{bi}")

                        for (cch, a, b, kind, glist) in planes:
                            dst = stg[(cch, a)][0:m, b : b + W - 1 : 2]
                            if kind == "ident":
                                nc.vector.tensor_copy(
                                    dst, E[a][0:m, 2 + b : 2 + b + W - 1 : 2]
                                )
                                continue
                            ps = ppool.tile([MBLK, NFREE], f32, tag="ps", name=f"ps{cch}{a}{b}_{img}_{bi}")
                            nmm = len(glist) * (2 if two_pass else 1)
                            i_mm = 0
                            for bmi, q, coff in glist:
                                rhs = Eh[q][:, coff : coff + W - 1 : 2]
                                nc.tensor.matmul(
                                    ps[0:m, :], bh[bmi][:, 0:m], rhs,
                                    start=(i_mm == 0), stop=(i_mm == nmm - 1),
                                )
                                i_mm += 1
                            if two_pass:
                                for bmi, q, coff in glist:
                                    rhs = El[q][:, coff : coff + W - 1 : 2]
                                    nc.tensor.matmul(
                                        ps[0:m, :], bl[bmi][:, 0:m], rhs,
                                        start=(i_mm == 0), stop=(i_mm == nmm - 1),
                                    )
                                    i_mm += 1
                            nc.scalar.copy(dst, ps[0:m, :])

                        for (cch, a) in stg:
                            r0 = 2 * i0 + a
                            nc.sync.dma_start(
                                y_d[img, cch, r0 : r0 + 2 * m - 1 : 2, :],
                                stg[(cch, a)][0:m, :],
                            )
    nc.compile()
    return nc


# ------------------------------------------------------------- SPMD runner
_CACHE = {}


def _get_compiled(index_key, planes, n_bmats, reps=1, two_pass=True):
    key = (index_key, reps, two_pass)
    if key not in _CACHE:
        _CACHE[key] = build_nc(planes, n_bmats, reps=reps, two_pass=two_pass)
    return _CACHE[key]


_RUNNER_CACHE = {}


def make_runner(nc, n_cores=N_CORES):
    """Cached jitted SPMD executor mirroring bass2jax.run_bass_via_pjrt's
    multi-core path, reusable across calls without re-tracing."""
    import jax
    import concourse.mybir as mybir_
    from concourse import bass2jax
    from jax.experimental.shard_map import shard_map
    from jax.sharding import Mesh, PartitionSpec

    bass2jax.install_neuronx_cc_hook()

    partition_name = (
        nc.partition_id_tensor.name if nc.partition_id_tensor else None
    )
    in_names, out_names, out_avals, zero_outs = [], [], [], []
    for alloc in nc.m.functions[0].allocations:
        if not isinstance(alloc, mybir_.MemoryLocationSet):
            continue
        name = alloc.memorylocations[0].name
        if alloc.kind == "ExternalInput":
            if name != partition_name:
                in_names.append(name)
        elif alloc.kind == "ExternalOutput":
            shape = tuple(alloc.tensor_shape)
            dtype = mybir_.dt.np(alloc.dtype)
            out_names.append(name)
            out_avals.append(jax.core.ShapedArray(shape, dtype))
            zero_outs.append(np.zeros(shape, dtype))
    n_params = len(in_names)
    n_outs = len(out_avals)
    all_in_names = in_names + out_names
    if partition_name is not None:
        all_in_names.append(partition_name)

    def _body(*args):
        operands = list(args)
        if partition_name is not None:
            operands.append(bass2jax.partition_id_tensor())
        outs = bass2jax._bass_exec_p.bind(
            *operands,
            out_avals=tuple(out_avals),
            in_names=tuple(all_in_names),
            out_names=tuple(out_names),
            lowering_input_output_aliases=(),
            sim_require_finite=True,
            sim_require_nnan=True,
            nc=nc,
        )
        return tuple(outs)

    devices = jax.devices()[:n_cores]
    mesh = Mesh(np.asarray(devices), ("core",))
    sharded = jax.jit(
        shard_map(
            _body, mesh=mesh,
            in_specs=(PartitionSpec("core"),) * (n_params + n_outs),
            out_specs=(PartitionSpec("core"),) * n_outs,
            check_rep=False,
        ),
        donate_argnums=tuple(range(n_params, n_params + n_outs)),
        keep_unused=True,
    )

    def run(in_maps):
        concat_in = [
            np.concatenate([np.asarray(m[name]) for m in in_maps], axis=0)
            for name in in_names
        ]
        concat_zeros = [
            np.zeros((n_cores * z.shape[0], *z.shape[1:]), z.dtype)
            for z in zero_outs
        ]
        out_arrs = sharded(*concat_in, *concat_zeros)
        return [
            {
                name: np.asarray(out_arrs[i]).reshape(
                    n_cores, *out_avals[i].shape
                )[c]
                for i, name in enumerate(out_names)
            }
            for c in range(n_cores)
        ]

    return run


def get_runner(reps=1, two_pass=True, index=None):
    if index is None:
        index = INDICES_RGGB
    index3 = np.asarray(index).reshape(3, 2, 2)
    ikey = tuple(index3.flatten().tolist())
    key = (ikey, reps, two_pass)
    if key not in _RUNNER_CACHE:
        planes, bmats = _build_plan(index3)
        nc = _get_compiled(ikey, planes, len(bmats), reps=reps, two_pass=two_pass)
        run = make_runner(nc)
        _RUNNER_CACHE[key] = (run, bmats)
    return _RUNNER_CACHE[key]


def kernel(x, bayer_pattern):
    x = np.ascontiguousarray(np.asarray(x), dtype=np.float32)
    bp = np.asarray(bayer_pattern)
    assert bp.reshape(-1, 4).shape[0] == 1, "per-batch bayer patterns unsupported"
    index = _calc_index(bp)
    run, bmats = get_runner(index=index)
    n = x.shape[0]
    xs = x.reshape(n, H, W)
    per = n // N_CORES
    bm16 = np.ascontiguousarray(bmats.astype(np.float16))
    bm32 = np.ascontiguousarray(bmats.astype(np.float32))
    in_maps = [
        {"x": xs[c * per : (c + 1) * per], "bm_h": bm16, "bm_l": bm32}
        for c in range(N_CORES)
    ]
    results = run(in_maps)
    out = np.concatenate([r["y"] for r in results], axis=0)
    return out.astype(np.float32)
